# revision 15
# baseline (speedup 1.0000x reference)
"""Trainium2 Bass kernel for the masked contrastive (MIL/NCE-style) loss.

Computes, for instance embeddings x[b,n,:], bag embeddings y[k,:]:
    logits[b,n,k] = cos(x[b,n], y[k]) / T
    loss = -mean_{mask}( logits[b,n,b] - logsumexp_{k!=b} logits[b,n,b] )

Strategy: data-parallel over bags across 8 NeuronCores (32 bags = 8192
instance rows per core). Every core receives the full bag embedding,
rolled by its bag offset so that each core's own-bag diagonal lands at a
fixed, compile-time-known column. Each core emits per-partition partial
sums of the masked per-instance terms and of the mask; the host does the
final (tiny) reduction and division.

v3 dataflow (per 128-row tile):
  - x loaded fp32 via HWDGE DMA issued from the otherwise-idle SP
    engine (frees GpSimd/Pool from SWDGE descriptor generation).
  - PE transposes x (f32r in, bf16 out, 1.5 cyc/row) into PSUM; the
    PSUM->SBUF fp8 cast copy is split between ScalarE and VectorE.
  - fp8 DoubleRow matmuls: Gram (row norms) + logits vs the fp8 bag.
  - row-norm ss extracted from the Gram diagonal with ONE fused DVE
    tensor_tensor_reduce (mult by identity, accumulate).
  - exp on ScalarE with per-row scale s = (1/T)/(SC*||x||), row-sum es
    via the ACT accumulator; the own-bag exp column e0 is copied out by
    the Pool engine ([P,1]); num/den are reconstructed at the end from
    e0/es in batched [P,NT] ops: num = ln e0, den = es - e0.
  - rsqrt is exp(-0.5*ln(ss)) so ScalarE only ever needs the
    natural_log_exp_and_others table set (one table load total).
"""

import os
import sys

import numpy as np

for _p in ("/opt/trn_rl_repo",):
    if os.path.isdir(_p) and _p not in sys.path:
        sys.path.append(_p)

B, N, D = 256, 256, 768
NCORES = 8
BPC = B // NCORES          # bags per core = 32
RPC = BPC * N              # instance rows per core = 8192
P = 128                    # partitions
NT = RPC // P              # row tiles per core = 64
DC = D // P                # contraction chunks = 6
K = B                      # logits columns = 256
EPS2 = 1e-16               # eps^2 for the norm clamp (eps = 1e-8)
LN2 = 0.6931471805599453   # ln(2) == ln(1/T) for T=0.5

_CACHE = {}


def _patch_act_tables():
    """Prefer the natural_log_exp_and_others ACT table set so Exp, Ln,
    Square and Copy all resolve to ONE resident table (the default
    first-match order picks exp_and_others for Exp and natural_log for
    Ln, reloading tables ~38x per kernel)."""
    import concourse.bacc as bacc
    import concourse.hw_specs as hw_specs

    if getattr(hw_specs, "_ct_patched", False):
        return
    orig = hw_specs.get_activation_tables

    def patched(module_arch):
        # IMPORTANT: set order (and therefore act_func_set_id indices) must
        # stay identical to act_info.json — walrus/NRT resolve the id by
        # file index.  So instead of reordering we hide Exp/Ln from every
        # other set, forcing the chooser onto the combined set.
        import concourse.mybir as mybir

        AF = mybir.ActivationFunctionType
        tabs = orig(module_arch)
        pref = "natural_log_exp_and_others"
        if pref not in tabs:
            return tabs
        return {
            name: (fns if name == pref else fns - {AF.Exp, AF.Ln})
            for name, fns in tabs.items()
        }

    hw_specs.get_activation_tables = patched
    hw_specs._ct_patched = True
    if getattr(bacc, "get_activation_tables", None) is orig:
        bacc.get_activation_tables = patched


def _build(repeat=1, cp_act=192, merge=8, xbufs=4, itbufs=3, scrbufs=3,
           group=4, tpbufs=2, grbufs=2, lgbufs=4, exbufs=3,
           loader="swdge", tdt="f32r", e0_eng="dve", es_mode="accum",
           diag="mulred", tail="e0", clamp_ss=False, compile_=True):
    """Build + compile the single-core SPMD program.

    cp_act: how many of the 768 transpose-copy columns go to ScalarE
    (rest to VectorE); merge: instance-row tiles loaded per DMA;
    loader: "hwdge" = fp32 loads issued from SP, "swdge" = casting
    bf16 loads issued from Pool (v1 behavior); tdt: transpose input
    interpretation ("f32r" 1.5 cyc/row, "f32" 2 cyc/row) for the hwdge
    loader; e0_eng: engine for the [P,1] own-bag exp column extract.
    """
    from contextlib import ExitStack

    import concourse.bacc as bacc
    import concourse.mybir as mybir
    import concourse.tile as tile
    from concourse.masks import make_identity

    _patch_act_tables()

    dt = mybir.dt
    AF = mybir.ActivationFunctionType
    ALU = mybir.AluOpType
    f32 = dt.float32
    f32r = dt.float32r
    bf16 = dt.bfloat16
    fp8 = dt.float8e4
    import math
    SC = 16.0  # fp8 pre-scale on normalized bag rows (folded into s)
    s_bias = LN2 - math.log(SC)

    nc = bacc.Bacc("TRN2", target_bir_lowering=False, debug=False,
                   num_devices=NCORES)
    inst = nc.dram_tensor("inst", [RPC, D], f32, kind="ExternalInput").ap()
    bag = nc.dram_tensor("bag", [K, D], f32, kind="ExternalInput").ap()
    maskT = nc.dram_tensor("maskT", [P, NT], dt.int32,
                           kind="ExternalInput").ap()
    out = nc.dram_tensor("out", [P, 2], f32, kind="ExternalOutput").ap()

    with tile.TileContext(nc) as tc, ExitStack() as ctx:
        consts = ctx.enter_context(tc.tile_pool(name="consts", bufs=1))
        xpool = ctx.enter_context(tc.tile_pool(name="x", bufs=xbufs))
        itpool = ctx.enter_context(tc.tile_pool(name="it", bufs=itbufs))
        scr = ctx.enter_context(tc.tile_pool(name="scr", bufs=scrbufs))
        expool = ctx.enter_context(tc.tile_pool(name="ex", bufs=exbufs))
        tp_ps = ctx.enter_context(tc.tile_pool(name="tp", bufs=tpbufs,
                                               space="PSUM"))
        gr_ps = ctx.enter_context(tc.tile_pool(name="gr", bufs=grbufs,
                                               space="PSUM"))
        lg_ps = ctx.enter_context(tc.tile_pool(name="lg", bufs=lgbufs,
                                               space="PSUM"))

        ident = consts.tile([P, P], f32)
        make_identity(nc, ident)
        ident_b = consts.tile([P, P], bf16)
        make_identity(nc, ident_b)

        zero_c = consts.tile([P, 1], f32)
        nc.vector.memset(zero_c, 0.0)
        ln2_c = consts.tile([P, 1], f32)
        nc.vector.memset(ln2_c, s_bias)

        mask_i = consts.tile([P, NT], dt.int32)
        nc.sync.dma_start(out=mask_i, in_=maskT)
        maskf = consts.tile([P, NT], f32)
        nc.gpsimd.tensor_copy(out=maskf, in_=mask_i)

        # ---- bag prep: bagnT[:, j*K:(j+1)*K] = SC * (bag_n^T)[d-chunk j] ----
        bagnT = consts.tile([P, DC * K], fp8)
        for kc in range(2):
            bXf = scr.tile([P, D], f32, tag="sq")
            nc.sync.dma_start(out=bXf, in_=bag[kc * P:(kc + 1) * P, :])
            bscr = scr.tile([P, D], f32, tag="sq2")
            bss = consts.tile([P, 1], f32, tag=f"bss{kc}")
            nc.scalar.activation(out=bscr, in_=bXf, func=AF.Square,
                                 bias=zero_c, accum_out=bss)
            nc.vector.tensor_scalar_max(bss, bss, EPS2)
            nc.scalar.activation(out=bss, in_=bss, func=AF.Ln, bias=zero_c)
            nc.scalar.activation(out=bss, in_=bss, func=AF.Exp, scale=-0.5,
                                 bias=zero_c)
            bX = xpool.tile([P, D], bf16, tag="bx")
            nc.vector.tensor_scalar(out=bX, in0=bXf, scalar1=bss,
                                    scalar2=None, op0=ALU.mult)
            tpb = tp_ps.tile([P, D], bf16, tag="tp")
            for j in range(DC):
                nc.tensor.transpose(tpb[:, j * P:(j + 1) * P],
                                    bX[:, j * P:(j + 1) * P], ident_b)
            for j in range(DC):
                dst = bagnT[:, j * K + kc * P: j * K + kc * P + P]
                nc.scalar.activation(out=dst, in_=tpb[:, j * P:(j + 1) * P],
                                     func=AF.Copy, scale=SC)

        ss_buf = consts.tile([P, NT], f32)
        sc2_buf = consts.tile([P, NT], f32)
        s_buf = consts.tile([P, NT], f32)
        es_buf = consts.tile([P, NT], f32)
        e0_buf = consts.tile([P, NT], f32)
        num_buf = den_buf = None
        if tail == "v1":
            num_buf = consts.tile([P, NT], f32, name="num_buf")
            den_buf = consts.tile([P, NT], f32, name="den_buf")

        inst3 = inst.rearrange("(t p) d -> t p d", p=P)
        x_tiles = {}
        xdt = bf16 if loader == "swdge" else f32

        def load_x(t):
            # `merge` row tiles per DMA call to amortize per-DMA overhead.
            if t in x_tiles:
                return x_tiles.pop(t)
            xm = xpool.tile([P, merge, D], xdt, tag="x")
            src = inst3[t:t + merge, :, :].rearrange("t p d -> p t d")
            if loader == "swdge":
                # SWDGE casting DMA (fp32 -> bf16 on the wire), Pool-issued
                nc.gpsimd.dma_start(out=xm, in_=src)
            else:
                # plain fp32 HWDGE DMA issued from the SP sequencer
                nc.sync.dma_start(out=xm, in_=src)
            for i in range(merge):
                x_tiles[t + i] = xm[:, i, :]
            return x_tiles.pop(t)

        e0eng = {"pool": nc.gpsimd, "dve": nc.vector,
                 "act": nc.scalar}[e0_eng]

        # Software-pipelined emission: per-engine queues execute in program
        # order, so each stage is emitted LAGGED so that by the time an
        # instruction reaches its queue head, its producers (on other
        # engines) have long finished:
        #   step t:   transposes(t) on PE        (X(t) prefetched by DMA)
        #   step t+1: copies(t) ACT/DVE, then matmuls(t) on PE
        #   step t+2: TTR(t) on DVE; rsqrt(group) on ACT at group end
        #   step t+5: exp(t) on ACT, e0(t)
        LAG_CP, LAG_TTR, LAG_EXP = 1, 2, 5
        DRmode = mybir.MatmulPerfMode.DoubleRow
        tin = f32r if tdt == "f32r" else f32

        for _rep in range(repeat):
            x_tiles.clear()
            st_tp = {}
            st_lg = {}
            lgp_tiles = {}
            for step in range(NT + LAG_EXP):
                t = step
                if t < NT:
                    X = load_x(t)
                    tp = tp_ps.tile([P, D], bf16, tag="tp")
                    if loader == "swdge":
                        for j in range(DC):
                            nc.tensor.transpose(tp[:, j * P:(j + 1) * P],
                                                X[:, j * P:(j + 1) * P],
                                                ident_b)
                    else:
                        for j in range(DC):
                            nc.tensor.transpose(
                                tp[:, j * P:(j + 1) * P],
                                X[:, j * P:(j + 1) * P].bitcast(tin),
                                ident.bitcast(tin))
                    st_tp[t] = tp

                t1 = step - LAG_CP
                if 0 <= t1 < NT:
                    tp = st_tp.pop(t1)
                    iT = itpool.tile([P, D], fp8, tag="it")
                    if cp_act > 0:
                        nc.scalar.copy(out=iT[:, :cp_act],
                                       in_=tp[:, :cp_act])
                    if cp_act < D:
                        nc.vector.tensor_copy(out=iT[:, cp_act:],
                                              in_=tp[:, cp_act:])
                    gr = gr_ps.tile([P, P], f32, tag="gr")
                    if t1 % 2 == 0:
                        lgp_tiles[t1 // 2] = lg_ps.tile(
                            [P, 2 * K], f32, tag="lg", name="lgp")
                    lg = lgp_tiles[t1 // 2][:, (t1 % 2) * K:
                                            (t1 % 2) * K + K]
                    for jp in range(DC // 2):
                        blk2 = iT[:, 2 * jp * P:(2 * jp + 2) * P].rearrange(
                            "p (two c) -> p two c", two=2)
                        bg2 = bagnT[:, 2 * jp * K:(2 * jp + 2) * K].rearrange(
                            "p (two k) -> p two k", two=2)
                        nc.tensor.matmul(gr, lhsT=blk2, rhs=blk2,
                                         start=(jp == 0),
                                         stop=(jp == DC // 2 - 1),
                                         perf_mode=DRmode)
                        nc.tensor.matmul(lg, lhsT=blk2, rhs=bg2,
                                         start=(jp == 0),
                                         stop=(jp == DC // 2 - 1),
                                         perf_mode=DRmode)
                    st_lg[t1] = (gr, lg)

                t2 = step - LAG_TTR
                if 0 <= t2 < NT:
                    gr, _ = st_lg[t2]
                    # ss[r] = gram diagonal, via one fused mult+reduce
                    gscr = scr.tile([P, P], f32, tag="gscr")
                    if diag == "ttr":
                        nc.vector.tensor_tensor_reduce(
                            out=gscr, in0=gr, in1=ident, scale=1.0,
                            scalar=0.0, op0=ALU.mult, op1=ALU.add,
                            accum_out=ss_buf[:, t2:t2 + 1])
                    else:
                        nc.vector.tensor_mul(gscr, gr, ident)
                        nc.vector.reduce_sum(ss_buf[:, t2:t2 + 1], gscr,
                                             axis=mybir.AxisListType.X)
                    if t2 % group == group - 1:
                        g = t2 // group
                        gsl = slice(g * group, (g + 1) * group)
                        # s = (1/T)/SC * rsqrt(ss)
                        #   = exp(-0.5*ln(ss) + ln2 - ln SC)
                        if clamp_ss:
                            nc.vector.tensor_scalar_max(ss_buf[:, gsl],
                                                        ss_buf[:, gsl], EPS2)
                        nc.scalar.activation(out=sc2_buf[:, gsl],
                                             in_=ss_buf[:, gsl],
                                             func=AF.Ln, bias=zero_c)
                        nc.scalar.activation(out=s_buf[:, gsl],
                                             in_=sc2_buf[:, gsl],
                                             func=AF.Exp, scale=-0.5,
                                             bias=ln2_c)

                t3 = step - LAG_EXP
                if 0 <= t3 < NT:
                    _, lg = st_lg.pop(t3)
                    b_col = t3 // 2  # own-bag column (bag rolled per core)
                    s_col = s_buf[:, t3:t3 + 1]
                    ex = expool.tile([P, K], f32, tag="ex")
                    if es_mode == "accum":
                        nc.scalar.activation(out=ex, in_=lg[:, 0:K],
                                             func=AF.Exp, scale=s_col,
                                             bias=zero_c,
                                             accum_out=es_buf[:, t3:t3 + 1])
                    else:
                        nc.scalar.activation(out=ex, in_=lg[:, 0:K],
                                             func=AF.Exp, scale=s_col,
                                             bias=zero_c)
                        if es_mode == "pool":
                            nc.gpsimd.reduce_sum(es_buf[:, t3:t3 + 1], ex,
                                                 axis=mybir.AxisListType.X)
                        else:  # "dve"
                            nc.vector.reduce_sum(es_buf[:, t3:t3 + 1], ex,
                                                 axis=mybir.AxisListType.X)
                    if tail == "e0":
                        e0eng.tensor_copy(out=e0_buf[:, t3:t3 + 1],
                                          in_=ex[:, b_col:b_col + 1])
                    else:
                        nc.vector.tensor_tensor(
                            out=num_buf[:, t3:t3 + 1],
                            in0=lg[:, b_col:b_col + 1],
                            in1=s_col, op=ALU.mult)
                        nc.vector.tensor_sub(den_buf[:, t3:t3 + 1],
                                             es_buf[:, t3:t3 + 1],
                                             ex[:, b_col:b_col + 1])

        # tail: num = ln(e0); den = es - e0; loss terms batched over [P,NT]
        outt = consts.tile([P, 2], f32)
        if tail == "e0":
            den = consts.tile([P, NT], f32)
            nc.vector.tensor_sub(den, es_buf, e0_buf)
            ld = consts.tile([P, NT], f32)
            nc.scalar.activation(out=ld, in_=den, func=AF.Ln, bias=zero_c)
            lnum = consts.tile([P, NT], f32)
            nc.scalar.activation(out=lnum, in_=e0_buf, func=AF.Ln,
                                 bias=zero_c)
            t1 = consts.tile([P, NT], f32)
            nc.vector.tensor_sub(t1, lnum, ld)
            nc.vector.tensor_mul(t1, t1, maskf)
            nc.vector.reduce_sum(outt[:, 0:1], t1,
                                 axis=mybir.AxisListType.X)
        else:
            ld = consts.tile([P, NT], f32)
            nc.scalar.activation(out=ld, in_=den_buf, func=AF.Ln,
                                 bias=zero_c)
            t1 = consts.tile([P, NT], f32)
            nc.vector.tensor_sub(t1, num_buf, ld)
            nc.vector.tensor_mul(t1, t1, maskf)
            nc.vector.reduce_sum(outt[:, 0:1], t1,
                                 axis=mybir.AxisListType.X)
        nc.vector.reduce_sum(outt[:, 1:2], maskf, axis=mybir.AxisListType.X)
        nc.sync.dma_start(out=out, in_=outt)

    if compile_:
        nc.compile()
    return nc


def _get(repeat=1, **kw):
    key = (repeat, tuple(sorted(kw.items())))
    if key not in _CACHE:
        _CACHE[key] = _build(repeat=repeat, **kw)
    return _CACHE[key]


def make_in_maps(instance_embedding, bag_embedding, mask):
    inst = np.ascontiguousarray(
        np.asarray(instance_embedding, dtype=np.float32).reshape(B * N, D))
    bagf = np.asarray(bag_embedding, dtype=np.float32)
    m = np.asarray(mask, dtype=np.int32).reshape(B * N)
    in_maps = []
    for c in range(NCORES):
        sh = inst[c * RPC:(c + 1) * RPC]
        bg = np.ascontiguousarray(np.roll(bagf, -c * BPC, axis=0))
        mt = np.ascontiguousarray(m[c * RPC:(c + 1) * RPC].reshape(NT, P).T)
        in_maps.append({"inst": sh, "bag": bg, "maskT": mt})
    return in_maps


def kernel(instance_embedding, bag_embedding, mask):
    from concourse import bass_utils

    nc = _get()
    in_maps = make_in_maps(instance_embedding, bag_embedding, mask)
    res = bass_utils.run_bass_kernel_spmd(nc, in_maps,
                                          core_ids=list(range(NCORES)))
    tsum = 0.0
    msum = 0.0
    for c in range(NCORES):
        o = res.results[c]["out"].astype(np.float64)
        tsum += o[:, 0].sum()
        msum += o[:, 1].sum()
    return np.array(-tsum / msum, dtype=np.float32)


if __name__ == "__main__":
    rng = np.random.default_rng(0)
    ie = rng.standard_normal((B, N, D), dtype=np.float32)
    be = rng.standard_normal((B, D), dtype=np.float32)
    mk = np.ones((B, N), dtype=np.int32)
    print("loss:", kernel(ie, be, mk))
